# revision 11
# baseline (speedup 1.0000x reference)
"""BertSelfAttention TRN2 kernel.

Problem: B=4, S=2048, H=768, NH=12, HD=64, fp32.
Sharding: 8 cores; core c owns batch b = c//2 and head-group g = c%2
(6 heads = 384 hidden cols). Each core runs the same Bass program on its
shard; host reassembles.

Per-core algorithm (all fp32):
  xT = x^T (PE transpose)                     [768, 2048]
  WT = W^T for q/k/v (PE transpose)           [768, 384]
  QT/KT = W @ xT + b   (pair layout: partitions [headA d | headB d])
  V = x @ WvT + bv, scaled by w_k = exp(mask_k), plus a w column
      (folds the additive attention mask into multiplicative row
       weights: softmax_k(s+m) = exp(s)w / sum_k exp(s)w)
  per (pair, q-quarter, kt-pair):
     scoresT[k,q] = K @ QT      (two heads row-packed on the PE, K=64 each)
     E = exp(SCALE * scoresT)   (ScalarE from PSUM, 1024-wide chunks)
     ctxT[d,q] and denom[q] accumulate in PSUM via v_aug = [v*w | w] (M=65)
  final: PE-transpose ctxT -> [q, d] and 1/denom -> [q, 1], multiply, DMA.
"""

import numpy as np

B, S, H = 4, 2048, 768
NH, HD = 12, 64
SCALE = 1.0 / np.sqrt(np.float32(HD)).astype(np.float32)
HPC = H // 2          # 384 hidden cols per core (6 heads)
NHEADS = 6            # heads per core
NPAIR = 3             # head pairs per core
NST = S // 128        # 16 S-tiles
NHT = H // 128        # 6 hidden tiles
NCORES = 8

_CACHE = {}


def build_nc(reps=1):
    import concourse.bacc as bacc
    import concourse.mybir as mybir
    import concourse.tile as tile
    from concourse.masks import make_identity

    f32 = mybir.dt.float32
    AF = mybir.ActivationFunctionType
    OP = mybir.AluOpType

    nc = bacc.Bacc("TRN2", target_bir_lowering=False, debug=False,
                   num_devices=NCORES)

    x_d = nc.declare_dram_parameter("x", [S, H], f32, isOutput=False)
    wq_d = nc.declare_dram_parameter("wq", [HPC, H], f32, isOutput=False)
    wk_d = nc.declare_dram_parameter("wk", [HPC, H], f32, isOutput=False)
    wv_d = nc.declare_dram_parameter("wv", [HPC, H], f32, isOutput=False)
    bq_d = nc.declare_dram_parameter("bq", [HPC], f32, isOutput=False)
    bk_d = nc.declare_dram_parameter("bk", [HPC], f32, isOutput=False)
    bv_d = nc.declare_dram_parameter("bv", [HPC], f32, isOutput=False)
    mask_d = nc.declare_dram_parameter("mask", [S], f32, isOutput=False)
    out_d = nc.declare_dram_parameter("out", [S, HPC], f32, isOutput=True)

    with tile.TileContext(nc) as tc:
        def body(_iv=None):
            import contextlib
            with contextlib.ExitStack() as stack:
                consts = stack.enter_context(tc.tile_pool(name="consts", bufs=1))
                p_qk = stack.enter_context(tc.tile_pool(name="p_qk", bufs=1))
                p_v = stack.enter_context(tc.tile_pool(name="p_v", bufs=1))
                p_e = stack.enter_context(tc.tile_pool(name="p_e", bufs=2))

                # ---- constants ----
                ident = consts.tile([128, 128], f32)
                make_identity(nc, ident)
                ones_row = consts.tile([1, 128], f32)
                nc.gpsimd.memset(ones_row, 1.0)
                ones6 = consts.tile([128, NHEADS], f32)
                nc.gpsimd.memset(ones6, 1.0)
                bq_sb = consts.tile([128, NPAIR], f32)
                nc.gpsimd.dma_start(out=bq_sb, in_=bq_d[:].rearrange("(t p) -> p t", p=128))
                bk_sb = consts.tile([128, NPAIR], f32)
                nc.gpsimd.dma_start(out=bk_sb, in_=bk_d[:].rearrange("(t p) -> p t", p=128))
                bv_sb = consts.tile([1, HPC], f32)
                nc.gpsimd.dma_start(out=bv_sb, in_=bv_d[:].rearrange("(o s) -> o s", o=1))
                mask_sb = consts.tile([128, NST], f32)
                nc.gpsimd.dma_start(out=mask_sb, in_=mask_d[:].rearrange("(t p) -> p t", p=128))
                # w = exp(mask): multiplicative fold of the additive mask
                wmask = consts.tile([128, NST], f32)
                nc.scalar.activation(wmask, mask_sb, AF.Exp)

                # ---- persistent (until end of attention) ----
                qt = p_qk.tile([128, NPAIR, S], f32)      # Q^T pairs
                kt_sb = p_qk.tile([128, NPAIR, S], f32)   # K^T pairs
                v_sb = p_v.tile([128, NST, NHEADS, HD + 1], f32)

                # ---- phase 1: transposes + QKV (xt/wt freed afterwards) ----
                with tc.tile_pool(name="p_xtw", bufs=1) as px, \
                        tc.tile_pool(name="psum_util", bufs=2, space="PSUM") as pu, \
                        tc.tile_pool(name="psum_qkv", bufs=1, space="PSUM") as psum_qkv:
                    xt = px.tile([128, NHT, S], f32)          # x^T
                    wt_q = px.tile([128, NHT, HPC], f32)      # Wq^T
                    wt_k = px.tile([128, NHT, HPC], f32)
                    wt_v = px.tile([128, NHT, HPC], f32)

                    for wd, wt in ((wq_d, wt_q), (wk_d, wt_k), (wv_d, wt_v)):
                        wtiles = []
                        for ot in range(3):
                            w_nat = px.tile([128, H], f32, tag="wnat", bufs=3)
                            nc.gpsimd.dma_start(out=w_nat, in_=wd[ot * 128:(ot + 1) * 128, :])
                            wtiles.append(w_nat)
                        for ht in range(NHT):
                            pw = pu.tile([128, HPC], f32, tag="tpw")
                            for ot in range(3):
                                nc.tensor.transpose(
                                    pw[:, ot * 128:(ot + 1) * 128],
                                    wtiles[ot][:, ht * 128:(ht + 1) * 128],
                                    ident)
                            nc.vector.tensor_copy(wt[:, ht, :], pw)
                    for stq in range(4):
                        xtiles = []
                        for c in range(4):
                            st = stq * 4 + c
                            x_nat = px.tile([128, H], f32, tag="xnat", bufs=6)
                            nc.gpsimd.dma_start(out=x_nat, in_=x_d[st * 128:(st + 1) * 128, :])
                            xtiles.append(x_nat)
                        for ht in range(NHT):
                            pxm = pu.tile([128, 512], f32, tag="tp1")
                            for c in range(4):
                                nc.tensor.transpose(
                                    pxm[:, c * 128:(c + 1) * 128],
                                    xtiles[c][:, ht * 128:(ht + 1) * 128],
                                    ident)
                            nc.vector.tensor_copy(xt[:, ht, stq * 512:(stq + 1) * 512], pxm)

                    # QKV projections
                    for wt, dst, bias in ((wt_q, qt, bq_sb), (wt_k, kt_sb, bk_sb)):
                        for pair in range(NPAIR):
                            for sc in range(4):
                                pq = psum_qkv.tile([128, 512], f32, tag="qt")
                                for ht in range(NHT):
                                    nc.tensor.matmul(
                                        pq,
                                        lhsT=wt[:, ht, pair * 128:(pair + 1) * 128],
                                        rhs=xt[:, ht, sc * 512:(sc + 1) * 512],
                                        start=(ht == 0), stop=(ht == NHT - 1))
                                nc.vector.tensor_scalar(
                                    out=dst[:, pair, sc * 512:(sc + 1) * 512],
                                    in0=pq, scalar1=bias[:, pair:pair + 1],
                                    scalar2=None, op0=OP.add)
                    for st in range(NST):
                        pv = psum_qkv.tile([128, HPC], f32, tag="v")
                        for ht in range(NHT):
                            nc.tensor.matmul(
                                pv,
                                lhsT=xt[:, ht, st * 128:(st + 1) * 128],
                                rhs=wt_v[:, ht, :],
                                start=(ht == 0), stop=False)
                        nc.tensor.matmul(pv, lhsT=ones_row, rhs=bv_sb,
                                         start=False, stop=True)
                        # v_aug = [v * w | w]
                        nc.vector.tensor_scalar(
                            out=v_sb[:, st, :, 0:HD],
                            in0=pv.rearrange("p (h d) -> p h d", h=NHEADS),
                            scalar1=wmask[:, st:st + 1], scalar2=None, op0=OP.mult)
                        nc.vector.tensor_scalar(
                            out=v_sb[:, st, :, HD], in0=ones6,
                            scalar1=wmask[:, st:st + 1], scalar2=None, op0=OP.mult)

                # ---- phase 2: attention ----
                # ctx_sb rows: [0:64] = even head of pair, [64:128] = odd head
                # den rows: head-local*4 + quarter
                with tc.tile_pool(name="p_ctx", bufs=1) as pc:
                    # row 64 of each block holds the softmax denominator
                    ctx_sa = pc.tile([65, NPAIR, 4, 512], f32)
                    ctx_sb2 = pc.tile([65, NPAIR, 4, 512], f32)
                    with tc.tile_pool(name="psum_attn", bufs=1, space="PSUM") as pa:
                        for pair in range(NPAIR):
                            for qtr in range(4):
                                qs = qtr * 512
                                ctx_a = pa.tile([65, 512], f32, tag="ctxa")
                                ctx_b = pa.tile([65, 512], f32, tag="ctxb")
                                for ktp in range(8):
                                    sa = pa.tile([128, 1024], f32, tag="sa")
                                    sb_ = pa.tile([128, 1024], f32, tag="sb")
                                    for par in range(2):
                                        kt = 2 * ktp + par
                                        nc.tensor.matmul(
                                            sa[:, par * 512:(par + 1) * 512],
                                            lhsT=kt_sb[0:64, pair, kt * 128:(kt + 1) * 128],
                                            rhs=qt[0:64, pair, qs:qs + 512],
                                            start=True, stop=True)
                                        nc.tensor.matmul(
                                            sb_[:, par * 512:(par + 1) * 512],
                                            lhsT=kt_sb[64:128, pair, kt * 128:(kt + 1) * 128],
                                            rhs=qt[64:128, pair, qs:qs + 512],
                                            start=True, stop=True)
                                    e_a = p_e.tile([128, 1024], f32, tag="ea")
                                    e_b = p_e.tile([128, 1024], f32, tag="eb")
                                    nc.scalar.activation(e_a, sa, AF.Exp, scale=float(SCALE))
                                    nc.scalar.activation(e_b, sb_, AF.Exp, scale=float(SCALE))
                                    for par in range(2):
                                        kt = 2 * ktp + par
                                        first = (ktp == 0 and par == 0)
                                        last = (ktp == 7 and par == 1)
                                        nc.tensor.matmul(
                                            ctx_a,
                                            lhsT=v_sb[:, kt, 2 * pair, :],
                                            rhs=e_a[:, par * 512:(par + 1) * 512],
                                            start=first, stop=last)
                                        nc.tensor.matmul(
                                            ctx_b,
                                            lhsT=v_sb[:, kt, 2 * pair + 1, :],
                                            rhs=e_b[:, par * 512:(par + 1) * 512],
                                            start=first, stop=last)
                                nc.vector.tensor_copy(
                                    ctx_sa[0:64, pair, qtr, :], ctx_a[0:64, :])
                                nc.vector.tensor_copy(
                                    ctx_sa[64:65, pair, qtr, :], ctx_a[64:65, :])
                                nc.vector.tensor_copy(
                                    ctx_sb2[0:64, pair, qtr, :], ctx_b[0:64, :])
                                nc.vector.tensor_copy(
                                    ctx_sb2[64:65, pair, qtr, :], ctx_b[64:65, :])

                    # ---- phase 3: transpose back, normalize, store ----
                    with tc.tile_pool(name="p_post", bufs=1) as po, \
                            tc.tile_pool(name="ppost", bufs=2, space="PSUM") as pp:
                        out_sb = po.tile([128, NST, HPC], f32)
                        for qtr in range(4):
                            for j in range(4):
                                st = qtr * 4 + j
                                for hl in range(NHEADS):
                                    pair, odd = hl // 2, hl % 2
                                    blk = ctx_sb2 if odd else ctx_sa
                                    tp = pp.tile([128, HD], f32,
                                                 tag="tpa" if hl % 2 == 0 else "tpb")
                                    nc.tensor.transpose(
                                        tp,
                                        blk[0:64, pair, qtr, j * 128:(j + 1) * 128],
                                        ident[0:64, 0:64])
                                    tpd = pp.tile([128, 1], f32,
                                                  tag="tpda" if hl % 2 == 0 else "tpdb")
                                    nc.tensor.transpose(
                                        tpd,
                                        blk[64:65, pair, qtr, j * 128:(j + 1) * 128],
                                        ident[64:65, 64:65])
                                    rec = po.tile([128, 1], f32, tag="rec", bufs=4)
                                    nc.vector.reciprocal(rec, tpd)
                                    nc.vector.tensor_scalar(
                                        out=out_sb[:, st, hl * HD:(hl + 1) * HD],
                                        in0=tp, scalar1=rec,
                                        scalar2=None, op0=OP.mult)
                                nc.gpsimd.dma_start(
                                    out=out_d[st * 128:(st + 1) * 128, :],
                                    in_=out_sb[:, st, :])

        if reps == 1:
            body()
        else:
            with tc.For_i(0, reps, 1) as iv:
                body(iv)

    nc.compile()
    return nc


def make_runner(nc):
    """jit-compiled shard_map runner over 8 cores."""
    import jax
    import numpy as _np
    from jax.sharding import Mesh, NamedSharding, PartitionSpec
    from jax.experimental.shard_map import shard_map
    import concourse.mybir as mybir
    from concourse.bass2jax import (_bass_exec_p, install_neuronx_cc_hook,
                                    partition_id_tensor)

    install_neuronx_cc_hook()
    part_name = nc.partition_id_tensor.name if nc.partition_id_tensor else None
    in_names, out_names, out_avals, out_shapes = [], [], [], []
    for alloc in nc.m.functions[0].allocations:
        if not isinstance(alloc, mybir.MemoryLocationSet):
            continue
        name = alloc.memorylocations[0].name
        if alloc.kind == "ExternalInput":
            if name != part_name:
                in_names.append(name)
        elif alloc.kind == "ExternalOutput":
            out_names.append(name)
            shape = tuple(alloc.tensor_shape)
            dtype = mybir.dt.np(alloc.dtype)
            out_avals.append(jax.core.ShapedArray(shape, dtype))
            out_shapes.append((shape, dtype))
    n_params = len(in_names)
    all_in_names = list(in_names) + list(out_names)
    if part_name is not None:
        all_in_names.append(part_name)

    def _body(*args):
        operands = list(args)
        if part_name is not None:
            operands.append(partition_id_tensor())
        outs = _bass_exec_p.bind(
            *operands,
            out_avals=tuple(out_avals),
            in_names=tuple(all_in_names),
            out_names=tuple(out_names),
            lowering_input_output_aliases=(),
            sim_require_finite=True,
            sim_require_nnan=True,
            nc=nc,
        )
        return tuple(outs)

    devices = jax.devices()[:NCORES]
    mesh = Mesh(_np.asarray(devices), ("core",))
    sharded = jax.jit(
        shard_map(_body, mesh=mesh,
                  in_specs=(PartitionSpec("core"),) * (n_params + len(out_names)),
                  out_specs=(PartitionSpec("core"),) * len(out_names),
                  check_rep=False),
        keep_unused=True)
    sh = NamedSharding(mesh, PartitionSpec("core"))

    def stage(in_maps):
        import jax as _jax
        concat_in = [
            _jax.device_put(
                _np.ascontiguousarray(
                    _np.concatenate([_np.asarray(m[name]) for m in in_maps], axis=0)),
                sh)
            for name in in_names]
        concat_zeros = [
            _jax.device_put(_np.zeros((NCORES * sh_[0], *sh_[1:]), dt), sh)
            for (sh_, dt) in out_shapes]
        return concat_in, concat_zeros

    def run(concat_in, concat_zeros):
        import jax as _jax
        outs = sharded(*concat_in, *concat_zeros)
        _jax.block_until_ready(outs)
        return outs

    def unpack(outs):
        res = []
        for c in range(NCORES):
            m = {}
            for i, name in enumerate(out_names):
                shape, dt = out_shapes[i]
                m[name] = np.asarray(outs[i]).reshape(NCORES, *shape)[c]
            res.append(m)
        return res

    return stage, run, unpack


def shard_inputs(hidden_states, attention_mask, Wq, bq, Wk, bk, Wv, bv):
    hs = np.asarray(hidden_states, dtype=np.float32)
    am = np.asarray(attention_mask, dtype=np.float32)
    Wq, Wk, Wv = (np.asarray(w, dtype=np.float32) for w in (Wq, Wk, Wv))
    bq, bk, bv = (np.asarray(b, dtype=np.float32) for b in (bq, bk, bv))
    in_maps = []
    for c in range(NCORES):
        b = c // 2
        g = c % 2
        rows = slice(g * HPC, (g + 1) * HPC)
        in_maps.append({
            "x": np.ascontiguousarray(hs[b]),
            "wq": np.ascontiguousarray(Wq[rows]),
            "wk": np.ascontiguousarray(Wk[rows]),
            "wv": np.ascontiguousarray(Wv[rows]),
            "bq": np.ascontiguousarray(bq[rows]),
            "bk": np.ascontiguousarray(bk[rows]),
            "bv": np.ascontiguousarray(bv[rows]),
            "mask": np.ascontiguousarray(am[b, 0, 0, :]),
        })
    return in_maps


def unshard_outputs(results):
    out = np.empty((B, S, H), dtype=np.float32)
    for c in range(NCORES):
        b = c // 2
        g = c % 2
        out[b][:, g * HPC:(g + 1) * HPC] = results[c]["out"]
    return out


def get_compiled(reps=1):
    if reps not in _CACHE:
        nc = build_nc(reps)
        stage, run, unpack = make_runner(nc)
        _CACHE[reps] = (nc, stage, run, unpack)
    return _CACHE[reps]


def kernel(hidden_states, attention_mask, Wq, bq, Wk, bk, Wv, bv):
    _, stage, run, unpack = get_compiled(reps=1)
    in_maps = shard_inputs(hidden_states, attention_mask, Wq, bq, Wk, bk, Wv, bv)
    ci, cz = stage(in_maps)
    outs = run(ci, cz)
    return unshard_outputs(unpack(outs))


# revision 13
# speedup vs baseline: 1.9933x; 1.9933x over previous
"""BertSelfAttention TRN2 kernel.

Problem: B=4, S=2048, H=768, NH=12, HD=64, fp32.
Sharding: 8 cores; core c owns batch b = c//2 and head-group g = c%2
(6 heads = 384 hidden cols). Each core runs the same Bass program on its
shard; host reassembles.

Per-core algorithm (all fp32):
  xT = x^T (PE transpose)                     [768, 2048]
  WT = W^T for q/k/v (PE transpose)           [768, 384]
  QT/KT = W @ xT + b   (pair layout: partitions [headA d | headB d])
  V = x @ WvT + bv, scaled by w_k = exp(mask_k), plus a w column
      (folds the additive attention mask into multiplicative row
       weights: softmax_k(s+m) = exp(s)w / sum_k exp(s)w)
  per (pair, q-quarter, kt-pair):
     scoresT[k,q] = K @ QT      (two heads row-packed on the PE, K=64 each)
     E = exp(SCALE * scoresT)   (ScalarE from PSUM, 1024-wide chunks)
     ctxT[d,q] and denom[q] accumulate in PSUM via v_aug = [v*w | w] (M=65)
  final: PE-transpose ctxT -> [q, d] and 1/denom -> [q, 1], multiply, DMA.
"""

import numpy as np

B, S, H = 4, 2048, 768
NH, HD = 12, 64
SCALE = 1.0 / np.sqrt(np.float32(HD)).astype(np.float32)
HPC = H // 2          # 384 hidden cols per core (6 heads)
NHEADS = 6            # heads per core
NPAIR = 3             # head pairs per core
NST = S // 128        # 16 S-tiles
NHT = H // 128        # 6 hidden tiles
NCORES = 8

_CACHE = {}


def build_nc(reps=1):
    import concourse.bacc as bacc
    import concourse.mybir as mybir
    import concourse.tile as tile
    from concourse.masks import make_identity

    f32 = mybir.dt.float32
    f32r = mybir.dt.float32r
    AF = mybir.ActivationFunctionType
    OP = mybir.AluOpType

    nc = bacc.Bacc("TRN2", target_bir_lowering=False, debug=False,
                   num_devices=NCORES)

    x_d = nc.declare_dram_parameter("x", [S, H], f32, isOutput=False)
    wq_d = nc.declare_dram_parameter("wq", [HPC, H], f32, isOutput=False)
    wk_d = nc.declare_dram_parameter("wk", [HPC, H], f32, isOutput=False)
    wv_d = nc.declare_dram_parameter("wv", [HPC, H], f32, isOutput=False)
    bq_d = nc.declare_dram_parameter("bq", [HPC], f32, isOutput=False)
    bk_d = nc.declare_dram_parameter("bk", [HPC], f32, isOutput=False)
    bv_d = nc.declare_dram_parameter("bv", [HPC], f32, isOutput=False)
    mask_d = nc.declare_dram_parameter("mask", [S], f32, isOutput=False)
    out_d = nc.declare_dram_parameter("out", [S, HPC], f32, isOutput=True)

    with tile.TileContext(nc) as tc:
        def body(_iv=None):
            import contextlib
            with contextlib.ExitStack() as stack:
                consts = stack.enter_context(tc.tile_pool(name="consts", bufs=1))
                p_qk = stack.enter_context(tc.tile_pool(name="p_qk", bufs=1))
                p_v = stack.enter_context(tc.tile_pool(name="p_v", bufs=1))
                p_e = stack.enter_context(tc.tile_pool(name="p_e", bufs=2))

                # ---- constants ----
                ident = consts.tile([128, 128], f32)
                make_identity(nc, ident)
                ones_row = consts.tile([1, 128], f32)
                nc.gpsimd.memset(ones_row, 1.0)
                ones6 = consts.tile([128, NHEADS], f32)
                nc.gpsimd.memset(ones6, 1.0)
                bq_sb = consts.tile([128, NPAIR], f32)
                nc.gpsimd.dma_start(out=bq_sb, in_=bq_d[:].rearrange("(t p) -> p t", p=128))
                bk_sb = consts.tile([128, NPAIR], f32)
                nc.gpsimd.dma_start(out=bk_sb, in_=bk_d[:].rearrange("(t p) -> p t", p=128))
                bv_sb = consts.tile([1, HPC], f32)
                nc.gpsimd.dma_start(out=bv_sb, in_=bv_d[:].rearrange("(o s) -> o s", o=1))
                mask_sb = consts.tile([128, NST], f32)
                nc.gpsimd.dma_start(out=mask_sb, in_=mask_d[:].rearrange("(t p) -> p t", p=128))
                # w = exp(mask): multiplicative fold of the additive mask
                wmask = consts.tile([128, NST], f32)
                nc.scalar.activation(wmask, mask_sb, AF.Exp)

                # ---- persistent (until end of attention) ----
                qt = p_qk.tile([128, NPAIR, S], f32r)      # Q^T pairs
                kt_sb = p_qk.tile([128, NPAIR, S], f32r)   # K^T pairs
                v_sb = p_v.tile([128, NST, NHEADS, HD + 1], f32r)

                # ---- phase 1: transposes + QKV (xt/wt freed afterwards) ----
                with tc.tile_pool(name="p_xtw", bufs=1) as px, \
                        tc.tile_pool(name="psum_util", bufs=2, space="PSUM") as pu, \
                        tc.tile_pool(name="psum_qkv", bufs=1, space="PSUM") as psum_qkv:
                    xt = px.tile([128, NHT, S], f32r)          # x^T
                    wt_q = px.tile([128, NHT, HPC], f32r)      # Wq^T
                    wt_k = px.tile([128, NHT, HPC], f32r)
                    wt_v = px.tile([128, NHT, HPC], f32r)

                    for wd, wt in ((wq_d, wt_q), (wk_d, wt_k), (wv_d, wt_v)):
                        wtiles = []
                        for ot in range(3):
                            w_nat = px.tile([128, H], f32, tag="wnat", bufs=3)
                            nc.gpsimd.dma_start(out=w_nat, in_=wd[ot * 128:(ot + 1) * 128, :])
                            wtiles.append(w_nat)
                        for ht in range(NHT):
                            pw = pu.tile([128, HPC], f32, tag="tpw")
                            for ot in range(3):
                                nc.tensor.transpose(
                                    pw[:, ot * 128:(ot + 1) * 128],
                                    wtiles[ot][:, ht * 128:(ht + 1) * 128],
                                    ident)
                            nc.vector.tensor_copy(wt[:, ht, :], pw)
                    for stq in range(4):
                        xtiles = []
                        for c in range(4):
                            st = stq * 4 + c
                            x_nat = px.tile([128, H], f32, tag="xnat", bufs=6)
                            nc.gpsimd.dma_start(out=x_nat, in_=x_d[st * 128:(st + 1) * 128, :])
                            xtiles.append(x_nat)
                        for ht in range(NHT):
                            pxm = pu.tile([128, 512], f32, tag="tp1")
                            for c in range(4):
                                nc.tensor.transpose(
                                    pxm[:, c * 128:(c + 1) * 128],
                                    xtiles[c][:, ht * 128:(ht + 1) * 128],
                                    ident)
                            nc.vector.tensor_copy(xt[:, ht, stq * 512:(stq + 1) * 512], pxm)

                    # QKV projections
                    for wt, dst, bias in ((wt_q, qt, bq_sb), (wt_k, kt_sb, bk_sb)):
                        for pair in range(NPAIR):
                            for sc in range(4):
                                pq = psum_qkv.tile([128, 512], f32, tag="qt")
                                for ht in range(NHT):
                                    nc.tensor.matmul(
                                        pq,
                                        lhsT=wt[:, ht, pair * 128:(pair + 1) * 128],
                                        rhs=xt[:, ht, sc * 512:(sc + 1) * 512],
                                        start=(ht == 0), stop=(ht == NHT - 1))
                                nc.vector.tensor_scalar(
                                    out=dst[:, pair, sc * 512:(sc + 1) * 512],
                                    in0=pq, scalar1=bias[:, pair:pair + 1],
                                    scalar2=None, op0=OP.add)
                    for st in range(NST):
                        pv = psum_qkv.tile([128, HPC], f32, tag="v")
                        for ht in range(NHT):
                            nc.tensor.matmul(
                                pv,
                                lhsT=xt[:, ht, st * 128:(st + 1) * 128],
                                rhs=wt_v[:, ht, :],
                                start=(ht == 0), stop=False)
                        nc.tensor.matmul(pv, lhsT=ones_row, rhs=bv_sb,
                                         start=False, stop=True)
                        # v_aug = [v * w | w]
                        nc.vector.tensor_scalar(
                            out=v_sb[:, st, :, 0:HD],
                            in0=pv.rearrange("p (h d) -> p h d", h=NHEADS),
                            scalar1=wmask[:, st:st + 1], scalar2=None, op0=OP.mult)
                        nc.vector.tensor_scalar(
                            out=v_sb[:, st, :, HD], in0=ones6,
                            scalar1=wmask[:, st:st + 1], scalar2=None, op0=OP.mult)

                # ---- phase 2: attention ----
                # ctx_sb rows: [0:64] = even head of pair, [64:128] = odd head
                # den rows: head-local*4 + quarter
                with tc.tile_pool(name="p_ctx", bufs=1) as pc:
                    # row 64 of each block holds the softmax denominator
                    ctx_sa = pc.tile([65, NPAIR, 4, 512], f32)
                    ctx_sb2 = pc.tile([65, NPAIR, 4, 512], f32)
                    with tc.tile_pool(name="psum_attn", bufs=1, space="PSUM") as pa:
                        for pair in range(NPAIR):
                            for qtr in range(4):
                                qs = qtr * 512
                                ctx_a = pa.tile([65, 512], f32, tag="ctxa")
                                ctx_b = pa.tile([65, 512], f32, tag="ctxb")
                                for ktp in range(8):
                                    sa = pa.tile([128, 1024], f32, tag="sa")
                                    sb_ = pa.tile([128, 1024], f32, tag="sb")
                                    for par in range(2):
                                        kt = 2 * ktp + par
                                        nc.tensor.matmul(
                                            sa[:, par * 512:(par + 1) * 512],
                                            lhsT=kt_sb[0:64, pair, kt * 128:(kt + 1) * 128],
                                            rhs=qt[0:64, pair, qs:qs + 512],
                                            start=True, stop=True)
                                        nc.tensor.matmul(
                                            sb_[:, par * 512:(par + 1) * 512],
                                            lhsT=kt_sb[64:128, pair, kt * 128:(kt + 1) * 128],
                                            rhs=qt[64:128, pair, qs:qs + 512],
                                            start=True, stop=True)
                                    e_a = p_e.tile([128, 1024], f32r, tag="ea")
                                    e_b = p_e.tile([128, 1024], f32r, tag="eb")
                                    nc.scalar.activation(e_a, sa, AF.Exp, scale=float(SCALE))
                                    nc.scalar.activation(e_b, sb_, AF.Exp, scale=float(SCALE))
                                    for par in range(2):
                                        kt = 2 * ktp + par
                                        first = (ktp == 0 and par == 0)
                                        last = (ktp == 7 and par == 1)
                                        nc.tensor.matmul(
                                            ctx_a,
                                            lhsT=v_sb[:, kt, 2 * pair, :],
                                            rhs=e_a[:, par * 512:(par + 1) * 512],
                                            start=first, stop=last)
                                        nc.tensor.matmul(
                                            ctx_b,
                                            lhsT=v_sb[:, kt, 2 * pair + 1, :],
                                            rhs=e_b[:, par * 512:(par + 1) * 512],
                                            start=first, stop=last)
                                nc.vector.tensor_copy(
                                    ctx_sa[0:64, pair, qtr, :], ctx_a[0:64, :])
                                nc.vector.tensor_copy(
                                    ctx_sa[64:65, pair, qtr, :], ctx_a[64:65, :])
                                nc.vector.tensor_copy(
                                    ctx_sb2[0:64, pair, qtr, :], ctx_b[0:64, :])
                                nc.vector.tensor_copy(
                                    ctx_sb2[64:65, pair, qtr, :], ctx_b[64:65, :])

                    # ---- phase 3: transpose back, normalize, store ----
                    with tc.tile_pool(name="p_post", bufs=1) as po, \
                            tc.tile_pool(name="ppost", bufs=2, space="PSUM") as pp:
                        out_sb = po.tile([128, NST, HPC], f32)
                        for qtr in range(4):
                            for j in range(4):
                                st = qtr * 4 + j
                                for hl in range(NHEADS):
                                    pair, odd = hl // 2, hl % 2
                                    blk = ctx_sb2 if odd else ctx_sa
                                    tp = pp.tile([128, HD], f32,
                                                 tag="tpa" if hl % 2 == 0 else "tpb")
                                    nc.tensor.transpose(
                                        tp,
                                        blk[0:64, pair, qtr, j * 128:(j + 1) * 128],
                                        ident[0:64, 0:64])
                                    tpd = pp.tile([128, 1], f32,
                                                  tag="tpda" if hl % 2 == 0 else "tpdb")
                                    nc.tensor.transpose(
                                        tpd,
                                        blk[64:65, pair, qtr, j * 128:(j + 1) * 128],
                                        ident[64:65, 64:65])
                                    rec = po.tile([128, 1], f32, tag="rec", bufs=4)
                                    nc.vector.reciprocal(rec, tpd)
                                    nc.vector.tensor_scalar(
                                        out=out_sb[:, st, hl * HD:(hl + 1) * HD],
                                        in0=tp, scalar1=rec,
                                        scalar2=None, op0=OP.mult)
                                nc.gpsimd.dma_start(
                                    out=out_d[st * 128:(st + 1) * 128, :],
                                    in_=out_sb[:, st, :])

        if reps == 1:
            body()
        else:
            with tc.For_i(0, reps, 1) as iv:
                body(iv)

    nc.compile()
    return nc


def make_runner(nc):
    """jit-compiled shard_map runner over 8 cores."""
    import jax
    import numpy as _np
    from jax.sharding import Mesh, NamedSharding, PartitionSpec
    from jax.experimental.shard_map import shard_map
    import concourse.mybir as mybir
    from concourse.bass2jax import (_bass_exec_p, install_neuronx_cc_hook,
                                    partition_id_tensor)

    install_neuronx_cc_hook()
    part_name = nc.partition_id_tensor.name if nc.partition_id_tensor else None
    in_names, out_names, out_avals, out_shapes = [], [], [], []
    for alloc in nc.m.functions[0].allocations:
        if not isinstance(alloc, mybir.MemoryLocationSet):
            continue
        name = alloc.memorylocations[0].name
        if alloc.kind == "ExternalInput":
            if name != part_name:
                in_names.append(name)
        elif alloc.kind == "ExternalOutput":
            out_names.append(name)
            shape = tuple(alloc.tensor_shape)
            dtype = mybir.dt.np(alloc.dtype)
            out_avals.append(jax.core.ShapedArray(shape, dtype))
            out_shapes.append((shape, dtype))
    n_params = len(in_names)
    all_in_names = list(in_names) + list(out_names)
    if part_name is not None:
        all_in_names.append(part_name)

    def _body(*args):
        operands = list(args)
        if part_name is not None:
            operands.append(partition_id_tensor())
        outs = _bass_exec_p.bind(
            *operands,
            out_avals=tuple(out_avals),
            in_names=tuple(all_in_names),
            out_names=tuple(out_names),
            lowering_input_output_aliases=(),
            sim_require_finite=True,
            sim_require_nnan=True,
            nc=nc,
        )
        return tuple(outs)

    devices = jax.devices()[:NCORES]
    mesh = Mesh(_np.asarray(devices), ("core",))
    sharded = jax.jit(
        shard_map(_body, mesh=mesh,
                  in_specs=(PartitionSpec("core"),) * (n_params + len(out_names)),
                  out_specs=(PartitionSpec("core"),) * len(out_names),
                  check_rep=False),
        keep_unused=True)
    sh = NamedSharding(mesh, PartitionSpec("core"))

    def stage(in_maps):
        import jax as _jax
        concat_in = [
            _jax.device_put(
                _np.ascontiguousarray(
                    _np.concatenate([_np.asarray(m[name]) for m in in_maps], axis=0)),
                sh)
            for name in in_names]
        concat_zeros = [
            _jax.device_put(_np.zeros((NCORES * sh_[0], *sh_[1:]), dt), sh)
            for (sh_, dt) in out_shapes]
        return concat_in, concat_zeros

    def run(concat_in, concat_zeros):
        import jax as _jax
        outs = sharded(*concat_in, *concat_zeros)
        _jax.block_until_ready(outs)
        return outs

    def unpack(outs):
        res = []
        for c in range(NCORES):
            m = {}
            for i, name in enumerate(out_names):
                shape, dt = out_shapes[i]
                m[name] = np.asarray(outs[i]).reshape(NCORES, *shape)[c]
            res.append(m)
        return res

    return stage, run, unpack


def shard_inputs(hidden_states, attention_mask, Wq, bq, Wk, bk, Wv, bv):
    hs = np.asarray(hidden_states, dtype=np.float32)
    am = np.asarray(attention_mask, dtype=np.float32)
    Wq, Wk, Wv = (np.asarray(w, dtype=np.float32) for w in (Wq, Wk, Wv))
    bq, bk, bv = (np.asarray(b, dtype=np.float32) for b in (bq, bk, bv))
    in_maps = []
    for c in range(NCORES):
        b = c // 2
        g = c % 2
        rows = slice(g * HPC, (g + 1) * HPC)
        in_maps.append({
            "x": np.ascontiguousarray(hs[b]),
            "wq": np.ascontiguousarray(Wq[rows]),
            "wk": np.ascontiguousarray(Wk[rows]),
            "wv": np.ascontiguousarray(Wv[rows]),
            "bq": np.ascontiguousarray(bq[rows]),
            "bk": np.ascontiguousarray(bk[rows]),
            "bv": np.ascontiguousarray(bv[rows]),
            "mask": np.ascontiguousarray(am[b, 0, 0, :]),
        })
    return in_maps


def unshard_outputs(results):
    out = np.empty((B, S, H), dtype=np.float32)
    for c in range(NCORES):
        b = c // 2
        g = c % 2
        out[b][:, g * HPC:(g + 1) * HPC] = results[c]["out"]
    return out


def get_compiled(reps=1):
    if reps not in _CACHE:
        nc = build_nc(reps)
        stage, run, unpack = make_runner(nc)
        _CACHE[reps] = (nc, stage, run, unpack)
    return _CACHE[reps]


def kernel(hidden_states, attention_mask, Wq, bq, Wk, bk, Wv, bv):
    _, stage, run, unpack = get_compiled(reps=1)
    in_maps = shard_inputs(hidden_states, attention_mask, Wq, bq, Wk, bk, Wv, bv)
    ci, cz = stage(in_maps)
    outs = run(ci, cz)
    return unshard_outputs(unpack(outs))


# revision 14
# speedup vs baseline: 6.4859x; 3.2539x over previous
"""BertSelfAttention TRN2 kernel.

Problem: B=4, S=2048, H=768, NH=12, HD=64, fp32.
Sharding: 8 cores; core c owns batch b = c//2 and head-group g = c%2
(6 heads = 384 hidden cols). Each core runs the same Bass program on its
shard; host reassembles.

Per-core algorithm (all fp32):
  xT = x^T (PE transpose)                     [768, 2048]
  WT = W^T for q/k/v (PE transpose)           [768, 384]
  QT/KT = W @ xT + b   (pair layout: partitions [headA d | headB d])
  V = x @ WvT + bv, scaled by w_k = exp(mask_k), plus a w column
      (folds the additive attention mask into multiplicative row
       weights: softmax_k(s+m) = exp(s)w / sum_k exp(s)w)
  per (pair, q-quarter, kt-pair):
     scoresT[k,q] = K @ QT      (two heads row-packed on the PE, K=64 each)
     E = exp(SCALE * scoresT)   (ScalarE from PSUM, 1024-wide chunks)
     ctxT[d,q] and denom[q] accumulate in PSUM via v_aug = [v*w | w] (M=65)
  final: PE-transpose ctxT -> [q, d] and 1/denom -> [q, 1], multiply, DMA.
"""

import numpy as np

B, S, H = 4, 2048, 768
NH, HD = 12, 64
SCALE = 1.0 / np.sqrt(np.float32(HD)).astype(np.float32)
HPC = H // 2          # 384 hidden cols per core (6 heads)
NHEADS = 6            # heads per core
NPAIR = 3             # head pairs per core
NST = S // 128        # 16 S-tiles
NHT = H // 128        # 6 hidden tiles
NCORES = 8

_CACHE = {}


def build_nc(reps=1):
    import concourse.bacc as bacc
    import concourse.mybir as mybir
    import concourse.tile as tile
    from concourse.masks import make_identity

    f32 = mybir.dt.float32
    f32r = mybir.dt.float32r
    bf16 = mybir.dt.bfloat16
    AF = mybir.ActivationFunctionType
    OP = mybir.AluOpType

    nc = bacc.Bacc("TRN2", target_bir_lowering=False, debug=False,
                   num_devices=NCORES)

    x_d = nc.declare_dram_parameter("x", [S, H], f32, isOutput=False)
    wq_d = nc.declare_dram_parameter("wq", [HPC, H], f32, isOutput=False)
    wk_d = nc.declare_dram_parameter("wk", [HPC, H], f32, isOutput=False)
    wv_d = nc.declare_dram_parameter("wv", [HPC, H], f32, isOutput=False)
    bq_d = nc.declare_dram_parameter("bq", [HPC], f32, isOutput=False)
    bk_d = nc.declare_dram_parameter("bk", [HPC], f32, isOutput=False)
    bv_d = nc.declare_dram_parameter("bv", [HPC], f32, isOutput=False)
    mask_d = nc.declare_dram_parameter("mask", [S], f32, isOutput=False)
    out_d = nc.declare_dram_parameter("out", [S, HPC], f32, isOutput=True)

    with tile.TileContext(nc) as tc:
        def body(_iv=None):
            import contextlib
            with contextlib.ExitStack() as stack:
                consts = stack.enter_context(tc.tile_pool(name="consts", bufs=1))
                p_qk = stack.enter_context(tc.tile_pool(name="p_qk", bufs=1))
                p_v = stack.enter_context(tc.tile_pool(name="p_v", bufs=1))
                p_e = stack.enter_context(tc.tile_pool(name="p_e", bufs=2))

                # ---- constants ----
                ident = consts.tile([128, 128], f32)
                make_identity(nc, ident)
                ones_row = consts.tile([1, 128], f32)
                nc.gpsimd.memset(ones_row, 1.0)
                ones6 = consts.tile([128, NHEADS], f32)
                nc.gpsimd.memset(ones6, 1.0)
                bq_sb = consts.tile([128, NPAIR], f32)
                nc.gpsimd.dma_start(out=bq_sb, in_=bq_d[:].rearrange("(t p) -> p t", p=128))
                bk_sb = consts.tile([128, NPAIR], f32)
                nc.gpsimd.dma_start(out=bk_sb, in_=bk_d[:].rearrange("(t p) -> p t", p=128))
                bv_sb = consts.tile([1, HPC], f32)
                nc.gpsimd.dma_start(out=bv_sb, in_=bv_d[:].rearrange("(o s) -> o s", o=1))
                mask_sb = consts.tile([128, NST], f32)
                nc.gpsimd.dma_start(out=mask_sb, in_=mask_d[:].rearrange("(t p) -> p t", p=128))
                # w = exp(mask): multiplicative fold of the additive mask
                wmask = consts.tile([128, NST], f32)
                nc.scalar.activation(wmask, mask_sb, AF.Exp)

                # ---- persistent (until end of attention) ----
                qt = p_qk.tile([128, NPAIR, S], bf16)      # Q^T pairs
                kt_sb = p_qk.tile([128, NPAIR, S], bf16)   # K^T pairs
                v_sb = p_v.tile([128, NST, NHEADS, HD + 1], f32r)

                # ---- phase 1: transposes + QKV (xt/wt freed afterwards) ----
                with tc.tile_pool(name="p_xtw", bufs=1) as px, \
                        tc.tile_pool(name="psum_util", bufs=2, space="PSUM") as pu, \
                        tc.tile_pool(name="psum_qkv", bufs=1, space="PSUM") as psum_qkv:
                    xt = px.tile([128, NHT, S], f32r)          # x^T
                    wt_q = px.tile([128, NHT, HPC], f32r)      # Wq^T
                    wt_k = px.tile([128, NHT, HPC], f32r)
                    wt_v = px.tile([128, NHT, HPC], f32r)

                    for wd, wt in ((wq_d, wt_q), (wk_d, wt_k), (wv_d, wt_v)):
                        wtiles = []
                        for ot in range(3):
                            w_nat = px.tile([128, H], f32, tag="wnat", bufs=3)
                            nc.gpsimd.dma_start(out=w_nat, in_=wd[ot * 128:(ot + 1) * 128, :])
                            wtiles.append(w_nat)
                        for ht in range(NHT):
                            pw = pu.tile([128, HPC], f32, tag="tpw")
                            for ot in range(3):
                                nc.tensor.transpose(
                                    pw[:, ot * 128:(ot + 1) * 128],
                                    wtiles[ot][:, ht * 128:(ht + 1) * 128],
                                    ident)
                            nc.vector.tensor_copy(wt[:, ht, :], pw)
                    for stq in range(4):
                        xtiles = []
                        for c in range(4):
                            st = stq * 4 + c
                            x_nat = px.tile([128, H], f32, tag="xnat", bufs=6)
                            nc.gpsimd.dma_start(out=x_nat, in_=x_d[st * 128:(st + 1) * 128, :])
                            xtiles.append(x_nat)
                        for ht in range(NHT):
                            pxm = pu.tile([128, 512], f32, tag="tp1")
                            for c in range(4):
                                nc.tensor.transpose(
                                    pxm[:, c * 128:(c + 1) * 128],
                                    xtiles[c][:, ht * 128:(ht + 1) * 128],
                                    ident)
                            nc.vector.tensor_copy(xt[:, ht, stq * 512:(stq + 1) * 512], pxm)

                    # QKV projections
                    for wt, dst, bias in ((wt_q, qt, bq_sb), (wt_k, kt_sb, bk_sb)):
                        for pair in range(NPAIR):
                            for sc in range(4):
                                pq = psum_qkv.tile([128, 512], f32, tag="qt")
                                for ht in range(NHT):
                                    nc.tensor.matmul(
                                        pq,
                                        lhsT=wt[:, ht, pair * 128:(pair + 1) * 128],
                                        rhs=xt[:, ht, sc * 512:(sc + 1) * 512],
                                        start=(ht == 0), stop=(ht == NHT - 1))
                                nc.vector.tensor_scalar(
                                    out=dst[:, pair, sc * 512:(sc + 1) * 512],
                                    in0=pq, scalar1=bias[:, pair:pair + 1],
                                    scalar2=None, op0=OP.add)
                    for st in range(NST):
                        pv = psum_qkv.tile([128, HPC], f32, tag="v")
                        for ht in range(NHT):
                            nc.tensor.matmul(
                                pv,
                                lhsT=xt[:, ht, st * 128:(st + 1) * 128],
                                rhs=wt_v[:, ht, :],
                                start=(ht == 0), stop=False)
                        nc.tensor.matmul(pv, lhsT=ones_row, rhs=bv_sb,
                                         start=False, stop=True)
                        # v_aug = [v * w | w]
                        nc.vector.tensor_scalar(
                            out=v_sb[:, st, :, 0:HD],
                            in0=pv.rearrange("p (h d) -> p h d", h=NHEADS),
                            scalar1=wmask[:, st:st + 1], scalar2=None, op0=OP.mult)
                        nc.vector.tensor_scalar(
                            out=v_sb[:, st, :, HD], in0=ones6,
                            scalar1=wmask[:, st:st + 1], scalar2=None, op0=OP.mult)

                # ---- phase 2: attention ----
                # ctx_sb rows: [0:64] = even head of pair, [64:128] = odd head
                # den rows: head-local*4 + quarter
                with tc.tile_pool(name="p_ctx", bufs=1) as pc:
                    # row 64 of each block holds the softmax denominator
                    ctx_sa = pc.tile([65, NPAIR, 4, 512], f32)
                    ctx_sb2 = pc.tile([65, NPAIR, 4, 512], f32)
                    with tc.tile_pool(name="psum_attn", bufs=1, space="PSUM") as pa:
                        for pair in range(NPAIR):
                            for qtr in range(4):
                                qs = qtr * 512
                                ctx_a = pa.tile([65, 512], f32, tag="ctxa")
                                ctx_b = pa.tile([65, 512], f32, tag="ctxb")
                                for ktp in range(8):
                                    sa = pa.tile([128, 1024], f32, tag="s", bufs=3)
                                    sb_ = pa.tile([128, 1024], f32, tag="s", bufs=3)
                                    for par in range(2):
                                        kt = 2 * ktp + par
                                        nc.tensor.matmul(
                                            sa[:, par * 512:(par + 1) * 512],
                                            lhsT=kt_sb[0:64, pair, kt * 128:(kt + 1) * 128],
                                            rhs=qt[0:64, pair, qs:qs + 512],
                                            start=True, stop=True)
                                        nc.tensor.matmul(
                                            sb_[:, par * 512:(par + 1) * 512],
                                            lhsT=kt_sb[64:128, pair, kt * 128:(kt + 1) * 128],
                                            rhs=qt[64:128, pair, qs:qs + 512],
                                            start=True, stop=True)
                                    e_a = p_e.tile([128, 1024], f32r, tag="ea")
                                    e_b = p_e.tile([128, 1024], f32r, tag="eb")
                                    nc.scalar.activation(e_a, sa, AF.Exp, scale=float(SCALE))
                                    nc.scalar.activation(e_b, sb_, AF.Exp, scale=float(SCALE))
                                    for par in range(2):
                                        kt = 2 * ktp + par
                                        first = (ktp == 0 and par == 0)
                                        last = (ktp == 7 and par == 1)
                                        nc.tensor.matmul(
                                            ctx_a,
                                            lhsT=v_sb[:, kt, 2 * pair, :],
                                            rhs=e_a[:, par * 512:(par + 1) * 512],
                                            start=first, stop=last)
                                        nc.tensor.matmul(
                                            ctx_b,
                                            lhsT=v_sb[:, kt, 2 * pair + 1, :],
                                            rhs=e_b[:, par * 512:(par + 1) * 512],
                                            start=first, stop=last)
                                nc.vector.tensor_copy(
                                    ctx_sa[0:64, pair, qtr, :], ctx_a[0:64, :])
                                nc.vector.tensor_copy(
                                    ctx_sa[64:65, pair, qtr, :], ctx_a[64:65, :])
                                nc.vector.tensor_copy(
                                    ctx_sb2[0:64, pair, qtr, :], ctx_b[0:64, :])
                                nc.vector.tensor_copy(
                                    ctx_sb2[64:65, pair, qtr, :], ctx_b[64:65, :])

                    # ---- phase 3: transpose back, normalize, store ----
                    with tc.tile_pool(name="p_post", bufs=1) as po, \
                            tc.tile_pool(name="ppost", bufs=2, space="PSUM") as pp:
                        out_sb = po.tile([128, NST, HPC], f32)
                        for qtr in range(4):
                            for j in range(4):
                                st = qtr * 4 + j
                                for hl in range(NHEADS):
                                    pair, odd = hl // 2, hl % 2
                                    blk = ctx_sb2 if odd else ctx_sa
                                    tp = pp.tile([128, HD], f32,
                                                 tag="tpa" if hl % 2 == 0 else "tpb")
                                    nc.tensor.transpose(
                                        tp,
                                        blk[0:64, pair, qtr, j * 128:(j + 1) * 128],
                                        ident[0:64, 0:64])
                                    tpd = pp.tile([128, 1], f32,
                                                  tag="tpda" if hl % 2 == 0 else "tpdb")
                                    nc.tensor.transpose(
                                        tpd,
                                        blk[64:65, pair, qtr, j * 128:(j + 1) * 128],
                                        ident[64:65, 64:65])
                                    rec = po.tile([128, 1], f32, tag="rec", bufs=4)
                                    nc.vector.reciprocal(rec, tpd)
                                    nc.vector.tensor_scalar(
                                        out=out_sb[:, st, hl * HD:(hl + 1) * HD],
                                        in0=tp, scalar1=rec,
                                        scalar2=None, op0=OP.mult)
                                nc.gpsimd.dma_start(
                                    out=out_d[st * 128:(st + 1) * 128, :],
                                    in_=out_sb[:, st, :])

        if reps == 1:
            body()
        else:
            with tc.For_i(0, reps, 1) as iv:
                body(iv)

    nc.compile()
    return nc


def make_runner(nc):
    """jit-compiled shard_map runner over 8 cores."""
    import jax
    import numpy as _np
    from jax.sharding import Mesh, NamedSharding, PartitionSpec
    from jax.experimental.shard_map import shard_map
    import concourse.mybir as mybir
    from concourse.bass2jax import (_bass_exec_p, install_neuronx_cc_hook,
                                    partition_id_tensor)

    install_neuronx_cc_hook()
    part_name = nc.partition_id_tensor.name if nc.partition_id_tensor else None
    in_names, out_names, out_avals, out_shapes = [], [], [], []
    for alloc in nc.m.functions[0].allocations:
        if not isinstance(alloc, mybir.MemoryLocationSet):
            continue
        name = alloc.memorylocations[0].name
        if alloc.kind == "ExternalInput":
            if name != part_name:
                in_names.append(name)
        elif alloc.kind == "ExternalOutput":
            out_names.append(name)
            shape = tuple(alloc.tensor_shape)
            dtype = mybir.dt.np(alloc.dtype)
            out_avals.append(jax.core.ShapedArray(shape, dtype))
            out_shapes.append((shape, dtype))
    n_params = len(in_names)
    all_in_names = list(in_names) + list(out_names)
    if part_name is not None:
        all_in_names.append(part_name)

    def _body(*args):
        operands = list(args)
        if part_name is not None:
            operands.append(partition_id_tensor())
        outs = _bass_exec_p.bind(
            *operands,
            out_avals=tuple(out_avals),
            in_names=tuple(all_in_names),
            out_names=tuple(out_names),
            lowering_input_output_aliases=(),
            sim_require_finite=True,
            sim_require_nnan=True,
            nc=nc,
        )
        return tuple(outs)

    devices = jax.devices()[:NCORES]
    mesh = Mesh(_np.asarray(devices), ("core",))
    sharded = jax.jit(
        shard_map(_body, mesh=mesh,
                  in_specs=(PartitionSpec("core"),) * (n_params + len(out_names)),
                  out_specs=(PartitionSpec("core"),) * len(out_names),
                  check_rep=False),
        keep_unused=True)
    sh = NamedSharding(mesh, PartitionSpec("core"))

    def stage(in_maps):
        import jax as _jax
        concat_in = [
            _jax.device_put(
                _np.ascontiguousarray(
                    _np.concatenate([_np.asarray(m[name]) for m in in_maps], axis=0)),
                sh)
            for name in in_names]
        concat_zeros = [
            _jax.device_put(_np.zeros((NCORES * sh_[0], *sh_[1:]), dt), sh)
            for (sh_, dt) in out_shapes]
        return concat_in, concat_zeros

    def run(concat_in, concat_zeros):
        import jax as _jax
        outs = sharded(*concat_in, *concat_zeros)
        _jax.block_until_ready(outs)
        return outs

    def unpack(outs):
        res = []
        for c in range(NCORES):
            m = {}
            for i, name in enumerate(out_names):
                shape, dt = out_shapes[i]
                m[name] = np.asarray(outs[i]).reshape(NCORES, *shape)[c]
            res.append(m)
        return res

    return stage, run, unpack


def shard_inputs(hidden_states, attention_mask, Wq, bq, Wk, bk, Wv, bv):
    hs = np.asarray(hidden_states, dtype=np.float32)
    am = np.asarray(attention_mask, dtype=np.float32)
    Wq, Wk, Wv = (np.asarray(w, dtype=np.float32) for w in (Wq, Wk, Wv))
    bq, bk, bv = (np.asarray(b, dtype=np.float32) for b in (bq, bk, bv))
    in_maps = []
    for c in range(NCORES):
        b = c // 2
        g = c % 2
        rows = slice(g * HPC, (g + 1) * HPC)
        in_maps.append({
            "x": np.ascontiguousarray(hs[b]),
            "wq": np.ascontiguousarray(Wq[rows]),
            "wk": np.ascontiguousarray(Wk[rows]),
            "wv": np.ascontiguousarray(Wv[rows]),
            "bq": np.ascontiguousarray(bq[rows]),
            "bk": np.ascontiguousarray(bk[rows]),
            "bv": np.ascontiguousarray(bv[rows]),
            "mask": np.ascontiguousarray(am[b, 0, 0, :]),
        })
    return in_maps


def unshard_outputs(results):
    out = np.empty((B, S, H), dtype=np.float32)
    for c in range(NCORES):
        b = c // 2
        g = c % 2
        out[b][:, g * HPC:(g + 1) * HPC] = results[c]["out"]
    return out


def get_compiled(reps=1):
    if reps not in _CACHE:
        nc = build_nc(reps)
        stage, run, unpack = make_runner(nc)
        _CACHE[reps] = (nc, stage, run, unpack)
    return _CACHE[reps]


def kernel(hidden_states, attention_mask, Wq, bq, Wk, bk, Wv, bv):
    _, stage, run, unpack = get_compiled(reps=1)
    in_maps = shard_inputs(hidden_states, attention_mask, Wq, bq, Wk, bk, Wv, bv)
    ci, cz = stage(in_maps)
    outs = run(ci, cz)
    return unshard_outputs(unpack(outs))
